# revision 14
# baseline (speedup 1.0000x reference)
"""Trainium2 Bass kernel for the Dirichlet-KDE ECE loss (nn_KDEECE).

reference math (N=8192, C=10, h=0.1):
  f        = softmax(logits)                      [N, C]
  alphas   = f/h + 1                              (sum_c alphas == C + 1/h == 20)
  log_beta = sum_c lgamma(alphas) - lgamma(20)    [N]
  t[j,i]   = log(f_i) . (10 f_j) - log_beta_j     (log kernel, diag excluded)
  kern     = exp(t)
  out      = mean_i sum_c | kern_y/den - f |,  kern_y = kern @ onehot,
             den = rowsum(kern)

Device strategy (8 cores, query rows i sharded, 1024 per core), v5:
  The ln matmul computes an AFFINE of the log kernel, P = A_SC*t + B_OFF,
  by scaling L and folding scale+offset into the [A | -lb] x [L | 1]
  augmentation.  Only ACT and DVE can read PSUM on TRN2, so exp is split
  across those two engines.  v5 uses ONE shared psum pool (3 bufs of
  [128,1024]) that both engines drain, one instruction per tile:

    ACT tiles: kern = Exp(P/A_SC - B_OFF/A_SC) as ONE 1024-wide
               activation -> bf16 (exact);  1038 ns/tile
    DVE tiles: u16 = uint16(max(P, 0)) as ONE 1024-wide tensor_scalar;
               kern = u16 viewed as bf16 (Schraudolph:
               P = 128*(log2(kern) + 127 + c) makes the uint16 bit
               pattern the bf16 encoding of ~exp(t));  1192 ns/tile
    PE : ky_ps[i 128, 10] += matmul(lhsT=kern[:, i-chunk], rhs=onehot[jt])
         (transposed accumulation: 10-wide output, nearly free)

  Tile->engine assignment is a Bresenham interleave of 34 ACT / 30 DVE
  (balances 34*1038 vs 30*1192).  Tiles 0..1 are processed as two
  512-wide reads so the stream starts one matmul earlier.  Input DMAs:
  lp rides the otherwise-idle Pool/SWDGE queue, ap's first 256 columns
  go first on the SP queue, so the first matmul fires ~350ns sooner.

  den comes from sum_c ky on the host (onehot columns partition j).
  The leave-one-out diagonal is subtracted on the host using a host-side
  replica of each tile's rounding (kii_dev).
"""

import numpy as np

N, C = 8192, 10
H_INV = 10.0
EPS_LOG = 1e-45
EPS_DEN = 1e-10
N_CORES = 8
LOC = N // N_CORES  # 1024 rows of i per core
JT = N // 128  # 64 j tiles

LOG2E = 1.4426950408889634
SCH_C = 0.043  # Schraudolph bias correction.  NOTE: hardware-validated
# optimum; do not host-tune this -- the device ACT Exp is a table-based
# approximation, so host emulation (exact exp) mispredicts the bias
# landscape (c=-0.04 predicted 4e-06 on host but measured 3.1e-04 on HW)
A_SC = 128.0 * LOG2E
B_OFF = 128.0 * (127.0 + SCH_C)

# perf knobs
KERN_BUFS = 12  # SBUF kern-tile buffering depth
PA_BUFS = 2  # ACT psum bufs [128, 1024]
PD_BUFS = 3  # DVE psum slots [128, 512]
PIPE_DEPTH = 4  # iterations between exp(jt) emission and ky(jt) emission
WARMUP = 2  # dummy PE matmuls before the loop (p-state ramp)
FILLERS = 0  # extra tiny PE matmuls before the stream (p-state burst);
# measured counterproductive: each costs ~213ns serially on the PE
ACT_TILE_NS = 1038.0  # cost-model busy per ACT exp tile
DVE_TILE_NS = 1316.0  # cost-model busy per DVE tile (2 halves)
SPLIT_JT = 63  # tile shared between engines to balance their finish times;
# placed LAST so the doubly-read psum tile never blocks a later refill
SPLIT_X = 832  # ACT takes cols [0:832] of tile 63, DVE takes [832:1024]
# (63 mod 8 == 7: that tile's leave-one-out diagonal columns [896:1024]
# sit inside the DVE piece, so the host kii replica treats it as DVE)

# kept for test.py compatibility (yone is already bf16 from _host_prep)
KY_BF16 = False


def _engine_of_jt():
    """Greedy static assignment of exp tiles to ACT(True)/DVE(False).
    SPLIT_JT is engine-shared on the device; its diagonal falls in the ACT
    piece so the replica treats it as ACT (True)."""
    assign = []
    busy_a, busy_d = 0.0, 0.0
    for jt in range(JT):
        if jt == SPLIT_JT:
            assign.append(False)  # replica: diagonal is in the DVE piece
            busy_a += 0.8333 * SPLIT_X + 185
            busy_d += 1.0417 * (LOC - SPLIT_X) + 125
            continue
        if busy_a + ACT_TILE_NS <= busy_d + DVE_TILE_NS:
            assign.append(True)
            busy_a += ACT_TILE_NS
        else:
            assign.append(False)
            busy_d += DVE_TILE_NS
    return assign


ENGINE_OF_JT = _engine_of_jt()

_compiled = None  # (nc, tensor names) cache across calls


def _lgamma(x):
    try:
        from scipy.special import gammaln

        return gammaln(x)
    except Exception:
        import math

        return np.vectorize(math.lgamma)(x.astype(np.float64))


def _schraudolph_bf16_f64(t_nats):
    """Host replica of the DVE tile function (float64 in/out)."""
    import ml_dtypes

    p = (A_SC * t_nats + B_OFF).astype(np.float32)
    p = np.maximum(p, 0.0)
    u = p.astype(np.uint16)  # C-cast truncation (matches interp)
    return u.view(ml_dtypes.bfloat16).astype(np.float64)


def _act_bf16_f64(t_nats):
    import ml_dtypes

    return np.exp(t_nats).astype(ml_dtypes.bfloat16).astype(np.float64)


def _host_prep(logits, labels):
    import ml_dtypes

    logits = np.asarray(logits, np.float32)
    labels = np.asarray(labels).astype(np.int64)
    x = logits - logits.max(axis=1, keepdims=True)
    e = np.exp(x)
    f = (e / e.sum(axis=1, keepdims=True)).astype(np.float32)

    f64 = f.astype(np.float64)
    alphas = f64 * H_INV + 1.0
    log_beta = _lgamma(alphas).sum(axis=1) - _lgamma(np.full(N, C + H_INV))
    L = np.log(f64 + EPS_LOG)
    A = (H_INV * f64).astype(np.float32)

    # apT = [A | -A_SC*lb + B_OFF]^T ; lpT = [A_SC*L | 1]^T  -> psum holds
    # P = A_SC * (L.A - lb) + B_OFF
    ap_last = (-A_SC * log_beta + B_OFF).astype(np.float32)
    apT = np.concatenate([A, ap_last[:, None]], axis=1).T.copy()  # [11, N]
    lp_rows = (A_SC * L).astype(np.float32)
    lpT = np.concatenate(
        [lp_rows, np.ones((N, 1), np.float32)], axis=1
    ).T.copy()  # [11, N]

    # onehot packed per j-tile: [128, 64*10] bf16 (no ones column; den from
    # the class sums)
    yone = np.zeros((N, C), np.float32)
    yone[np.arange(N), labels] = 1.0
    yone_packed = (
        yone.reshape(JT, 128, C)
        .transpose(1, 0, 2)
        .reshape(128, JT * C)
        .astype(ml_dtypes.bfloat16)
        .copy()
    )

    # device self-term: replicate each diagonal tile's rounding exactly
    t_ii = (L * (H_INV * f64)).sum(axis=1) - log_beta
    kii = np.empty(N, np.float64)
    act_mask = np.array(ENGINE_OF_JT)[np.arange(N) // 128]
    kii[act_mask] = _act_bf16_f64(t_ii[act_mask])
    kii[~act_mask] = _schraudolph_bf16_f64(t_ii[~act_mask])
    return f, labels, apT, lpT, yone_packed, kii


def _build():
    import concourse.bacc as bacc
    import concourse.mybir as mybir
    import concourse.tile as tile

    f32 = mybir.dt.float32
    f32r = mybir.dt.float32r
    bf16 = mybir.dt.bfloat16
    u16 = mybir.dt.uint16
    nc = bacc.Bacc(
        "TRN2",
        target_bir_lowering=False,
        debug=False,
        enable_asserts=False,
        num_devices=N_CORES,
    )
    ap_d = nc.dram_tensor("apT", [11, N], f32r, kind="ExternalInput")
    lp_d = nc.dram_tensor("lpT", [11, LOC], f32r, kind="ExternalInput")
    yo_d = nc.dram_tensor("yone", [128, JT * C], bf16, kind="ExternalInput")
    ky_d = nc.dram_tensor("ky", [128, 8 * C], f32, kind="ExternalOutput")

    # ap DMA pieces: chunk 0 gates the stream start, rest arrives while
    # the early tiles stream
    ap_cuts = [0, 2048, 4096, 6144, N]

    with tile.TileContext(nc) as tc:
        with (
            tc.tile_pool(name="const", bufs=1) as cp,
            tc.tile_pool(name="kern", bufs=KERN_BUFS) as kp,
            tc.tile_pool(name="pa", bufs=PA_BUFS, space="PSUM") as pap,
            tc.tile_pool(name="pd", bufs=PD_BUFS, space="PSUM") as pdp,
            tc.tile_pool(name="kyp", bufs=1, space="PSUM") as kyp,
        ):
            ap_sb = cp.tile([11, N], f32r)
            lp_sb = cp.tile([11, LOC], f32r)
            yo_sb = cp.tile([128, JT * C], bf16)
            wu_sb = cp.tile([11, 64], f32)
            bias_sb = cp.tile([128, 1], f32)
            scr_sb = cp.tile([128, 1], f32)

            # DMA first.  lp gates the first matmul; ap chunk 0 next; yone
            # is needed only at the first ky (PIPE_DEPTH tiles in).
            # NOTE: an earlier lp arrival (Pool/SWDGE queue, ~3.4us) was
            # tried and REGRESSED: the cost model's PE p-state ramp keeps
            # matmuls at half speed until ~3.8us, so the earlier stream
            # start only starves the exp engines.
            nc.scalar.dma_start(lp_sb[:], lp_d.ap())
            nc.sync.dma_start(
                ap_sb[:, 0 : ap_cuts[1]], ap_d.ap()[:, 0 : ap_cuts[1]]
            )
            nc.sync.dma_start(yo_sb[:], yo_d.ap())
            for lo, hi in zip(ap_cuts[1:-1], ap_cuts[2:]):
                nc.sync.dma_start(ap_sb[:, lo:hi], ap_d.ap()[:, lo:hi])
            # warmup constants on DVE (free until the stream starts)
            nc.vector.memset(wu_sb[:], 0.0)
            nc.vector.memset(bias_sb[:], -B_OFF / A_SC)
            # pull the one-time Exp table load out of the steady state
            nc.scalar.activation(
                scr_sb[:], bias_sb[:], mybir.ActivationFunctionType.Exp
            )

            ky_ps = kyp.tile([128, 512], f32)

            # tiny PE matmuls so pe_busy_start latches early (p-state ramp);
            # target the unused tail of the ky bank.  The FILLERS keep the
            # PE busy-burst alive until the lp DMA lands so the real stream
            # starts at (or near) full clock.
            for w in range(WARMUP + FILLERS):
                nc.tensor.matmul(
                    ky_ps[0:64, 448:512], wu_sb[:], wu_sb[:],
                    start=True, stop=True,
                )

            kern_tiles = [None] * JT
            for jt in range(JT + PIPE_DEPTH):
                # ky batch first: its deps are long met, so it flows through
                # the PE wait queue without head-of-line blocking the
                # latency-critical ln matmuls below.
                if jt >= PIPE_DEPTH:
                    p = jt - PIPE_DEPTH
                    yw = yo_sb[:, p * C : (p + 1) * C]
                    kprev = kern_tiles[p]
                    for h in range(8):
                        nc.tensor.matmul(
                            ky_ps[:, h * C : (h + 1) * C],
                            kprev[:, h * 128 : (h + 1) * 128],
                            yw,
                            start=(p == 0),
                            stop=(p == JT - 1),
                        )
                    kern_tiles[p] = None
                if jt < JT:
                    w = ap_sb[:, jt * 128 : (jt + 1) * 128]
                    k_sb = kp.tile([128, LOC], bf16)
                    if jt == SPLIT_JT:
                        # engine-shared tile: ACT cols [0:SPLIT_X], DVE rest
                        ln_ps = pap.tile([128, LOC], f32, tag="pa_ps")
                        for h in range(2):
                            nc.tensor.matmul(
                                ln_ps[:, h * 512 : (h + 1) * 512],
                                w,
                                lp_sb[:, h * 512 : (h + 1) * 512],
                                start=True,
                                stop=True,
                            )
                        nc.scalar.activation(
                            k_sb[:, 0:SPLIT_X],
                            ln_ps[:, 0:SPLIT_X],
                            mybir.ActivationFunctionType.Exp,
                            bias=bias_sb[:],
                            scale=1.0 / A_SC,
                        )
                        nc.vector.tensor_scalar(
                            k_sb[:, SPLIT_X:LOC].bitcast(u16),
                            ln_ps[:, SPLIT_X:LOC],
                            0.0,
                            None,
                            op0=mybir.AluOpType.max,
                        )
                    elif ENGINE_OF_JT[jt]:
                        ln_ps = pap.tile([128, LOC], f32, tag="pa_ps")
                        for h in range(2):
                            nc.tensor.matmul(
                                ln_ps[:, h * 512 : (h + 1) * 512],
                                w,
                                lp_sb[:, h * 512 : (h + 1) * 512],
                                start=True,
                                stop=True,
                            )
                        nc.scalar.activation(
                            k_sb[:],
                            ln_ps[:],
                            mybir.ActivationFunctionType.Exp,
                            bias=bias_sb[:],
                            scale=1.0 / A_SC,
                        )
                    else:
                        for h in range(2):
                            sl = pdp.tile([128, 512], f32, tag="pd_ps")
                            nc.tensor.matmul(
                                sl[:],
                                w,
                                lp_sb[:, h * 512 : (h + 1) * 512],
                                start=True,
                                stop=True,
                            )
                            nc.vector.tensor_scalar(
                                k_sb[:, h * 512 : (h + 1) * 512].bitcast(u16),
                                sl[:],
                                0.0,
                                None,
                                op0=mybir.AluOpType.max,
                            )
                    kern_tiles[jt] = k_sb

            out_sb = cp.tile([128, 8 * C], f32)
            nc.vector.tensor_copy(out_sb[:], ky_ps[:, 0 : 8 * C])
            nc.sync.dma_start(ky_d.ap(), out_sb[:])

    nc.compile()
    return nc, ap_d.name, lp_d.name, yo_d.name, ky_d.name


def kernel(logits, labels):
    global _compiled
    from concourse import bass_utils

    f, labels_i, apT, lpT, yone_packed, kii = _host_prep(logits, labels)

    if _compiled is None:
        _compiled = _build()
    nc, ap_name, lp_name, yo_name, ky_name = _compiled

    in_maps = []
    for d in range(N_CORES):
        in_maps.append(
            {
                ap_name: apT,
                lp_name: lpT[:, d * LOC : (d + 1) * LOC].copy(),
                yo_name: yone_packed,
            }
        )
    res = bass_utils.run_bass_kernel_spmd(nc, in_maps, core_ids=list(range(N_CORES)))
    # per-core [128, 8*C]: partition p, col h*C+c -> i = d*LOC + h*128 + p
    ky = np.concatenate(
        [
            res.results[d][ky_name]
            .reshape(128, 8, C)
            .transpose(1, 0, 2)
            .reshape(LOC, C)
            for d in range(N_CORES)
        ],
        axis=0,
    ).astype(np.float64)  # [N, C], includes diagonal

    den = ky.sum(axis=1) - kii
    ky[np.arange(N), labels_i] -= kii
    den = np.maximum(den, EPS_DEN)
    ratio = ky / den[:, None]
    per_sample = np.abs(ratio - f.astype(np.float64)).sum(axis=1)
    return np.asarray(per_sample.mean(), dtype=np.float32)
